# revision 1
# baseline (speedup 1.0000x reference)
"""Trainium2 Bass kernel for nn_Attention_3126736192307.

Causal multi-head attention with RoPE: B=2, S=2048, H=2048, 16 heads x 128.

Sharding (tensor parallel over heads, 8 cores, 2 heads each):
  - Wq/Wk/Wv column-split (per-head), Wo row-split; each core computes a
    partial [B*S, H] output; the host sums the 8 partials (row-parallel
    unshard) - no on-device collectives needed.

Per-core dataflow (all matmuls transpose-free by construction):
  - Host pre-transposes: X.T [H, T], WqT/WkT [H, 256] (head-dim permuted so
    RoPE's rotate_half becomes an intra-quadrant stream_shuffle), WvT [H, 256],
    WoT [256, H], cos/sin [128, T] feature-major (sin sign-folded).
  - Phase 1: q,k feature-major [128, T] per head + RoPE (DVE); v token-major.
  - Phase 2 per (b, h, i-chunk): scores.T [j,i] = k.T (lhsT) @ q.T; exp on
    ScalarE (no max subtraction - scores are ~N(0,1) after the 1/sqrt(hd)
    scale); causal block skipping + 0/1 mask multiply on diagonal-crossing
    tiles; column sums via ones-matmul on TensorE; AV accumulation in PSUM;
    normalization folded into the PSUM->SBUF eviction.
  - Phase 3: out.T (lhsT) @ WoT -> partial [T, H], streamed to DRAM.

Matmuls run in bf16 (1 PE cycle/row; fp32 is 4x, and fp32r's fused
weight-load encoding can't carry the 2 semaphore waits Tile emits).
"""

import os
import sys

for _p in ("/opt/trn_rl_repo", "/root/.axon_site/_ro/trn_rl_repo"):
    if os.path.isdir(_p) and _p not in sys.path:
        sys.path.append(_p)

from contextlib import ExitStack

import ml_dtypes
import numpy as np

import concourse.bass as bass
import concourse.bacc as bacc
import concourse.tile as tile
from concourse import mybir
from concourse.bass_utils import run_bass_kernel_spmd

B, S, H, NH = 2, 2048, 2048, 16
HD = 128
NCORES = 8
HPC = NH // NCORES            # heads per core = 2
M = HPC * HD                  # 256 output channels per core
SCALE = HD ** -0.5
P = 128                       # partitions
NKT = H // P                  # 16 contraction tiles for projections

F32 = mybir.dt.float32

# head-dim permutation: interleave halves at 16 granularity so the RoPE
# partner (d <-> d+64) sits 16 partitions away inside one 32-part quadrant
PERM = np.concatenate([np.arange(16 * m, 16 * m + 16) + (64 if odd else 0)
                       for m in range(4) for odd in (0, 1)])
SWAP_MASK = [i ^ 16 for i in range(32)]


BF16 = ml_dtypes.bfloat16


def build_masks(tchunk):
    """0/1 keep-masks for the R diagonal-crossing j-tiles of each i-chunk."""
    r = tchunk // P
    m = np.zeros((r, P, tchunk), np.float32)
    il = np.arange(tchunk)
    for ri in range(r):
        for jl in range(P):
            m[ri, jl, :] = (P * ri + jl <= il).astype(np.float32)
    return m


def build_nc(s=S, b=B, tchunk=512, mm_dtype=mybir.dt.bfloat16):
    t = b * s
    tchunk = min(tchunk, t)
    nch = t // tchunk             # phase-1 token chunks
    ich = s // tchunk             # attention i-chunks per batch
    r_mask = tchunk // P          # diagonal-crossing tiles per i-chunk
    ntt = t // P                  # token tiles

    FR = mm_dtype

    def mm(ap):
        return ap

    nc = bacc.Bacc("TRN2", target_bir_lowering=False, debug=False)

    xt = nc.declare_dram_parameter("xt", [H, t], FR, isOutput=False)
    wqt = nc.declare_dram_parameter("wqt", [H, M], FR, isOutput=False)
    wkt = nc.declare_dram_parameter("wkt", [H, M], FR, isOutput=False)
    wvt = nc.declare_dram_parameter("wvt", [H, M], FR, isOutput=False)
    wot = nc.declare_dram_parameter("wot", [M, H], FR, isOutput=False)
    cost = nc.declare_dram_parameter("cost", [HD, t], F32, isOutput=False)
    sint = nc.declare_dram_parameter("sint", [HD, t], F32, isOutput=False)
    masks = nc.declare_dram_parameter("masks", [r_mask, P, tchunk], FR,
                                      isOutput=False)
    out = nc.declare_dram_parameter("out", [t, H], FR, isOutput=True)

    with tile.TileContext(nc) as tc, ExitStack() as ctx:
        persist = ctx.enter_context(tc.tile_pool(name="persist", bufs=1))

        # persistent activations
        qr = [persist.tile([P, t], FR, tag=f"qr{h}", name=f"qr{h}") for h in range(HPC)]
        kr = [persist.tile([P, t], FR, tag=f"kr{h}", name=f"kr{h}") for h in range(HPC)]
        vv = persist.tile([P, ntt, M], FR, tag="vv")   # v[tt*128+p, d]
        ones_s = persist.tile([P, P], FR, tag="ones")
        nc.vector.memset(ones_s[:], 1.0)
        # allocated up-front (fresh SBUF -> no reuse waits on their DMAs);
        # loads issued after phase 1 so they don't delay the first matmuls
        mask_s = persist.tile([P, r_mask, tchunk], FR, tag="masks")
        wo_s = persist.tile([P, HPC, H], FR, tag="wo")
        ev_pool = ctx.enter_context(tc.tile_pool(name="evp", bufs=8))
        # whole-kernel 2-bank PSUM tiles: phase-1 q/k accumulator pairs and
        # attention score tiles rotate through the same two slots (A, B) --
        # no pool-handoff barrier on the critical QK path
        ab_pool = ctx.enter_context(tc.tile_pool(name="ab", bufs=1, space="PSUM"))

        # ---------------- phase 1: projections + rope -----------------
        with (
            tc.tile_pool(name="csin", bufs=2) as csin_pool,
            tc.tile_pool(name="xtp", bufs=3) as xt_pool,
            tc.tile_pool(name="rtmp", bufs=3) as rtmp_pool,
            tc.tile_pool(name="wts", bufs=1) as wts_pool,
            tc.tile_pool(name="p1v", bufs=1, space="PSUM") as p1v,
        ):
            wq_s = wts_pool.tile([P, NKT, M], FR, tag="wq")
            wk_s = wts_pool.tile([P, NKT, M], FR, tag="wk")
            wv_s = wts_pool.tile([P, NKT, M], FR, tag="wv")
            KG = 4                       # k-tiles per DMA
            for c in range(nch):
                tsl = slice(c * tchunk, (c + 1) * tchunk)
                cos_t = csin_pool.tile([P, tchunk], F32, tag="cos")
                sin_t = csin_pool.tile([P, tchunk], F32, tag="sin")

                # kt-outer: each X.T k-tile feeds all 8 accumulators, then dies
                q_ps = ab_pool.tile([P, HPC, 512], F32, tag="A", name=f"qps_{c}")
                k_ps = ab_pool.tile([P, HPC, 512], F32, tag="B", name=f"kps_{c}")
                qk_ps = [q_ps[:, 0, :tchunk], q_ps[:, 1, :tchunk],
                         k_ps[:, 0, :tchunk], k_ps[:, 1, :tchunk]]
                nvp = tchunk // P
                v_ps = [p1v.tile([P, M], F32, tag=f"p1v{i}",
                                 name=f"p1v{i}_{c}") for i in range(nvp)]
                for g in range(NKT // KG):
                    gsl = slice(g * KG * P, (g + 1) * KG * P)
                    if c == 0:
                        # weight loads on the (otherwise idle) gpsimd SWDGE
                        # queue: they issue in parallel with SP's x loads
                        for w_s, wsrc in ((wq_s, wqt), (wk_s, wkt),
                                          (wv_s, wvt)):
                            nc.gpsimd.dma_start(
                                out=w_s[:, g * KG:(g + 1) * KG, :],
                                in_=wsrc[gsl, :].rearrange(
                                    "(k p) m -> p k m", p=P))
                    xt4 = xt_pool.tile([P, KG, tchunk], FR, tag="xt")
                    nc.sync.dma_start(
                        out=xt4[:],
                        in_=xt[gsl, tsl].rearrange("(k p) t -> p k t", p=P))
                    for kk in range(KG):
                        kt = g * KG + kk
                        fl = dict(start=(kt == 0), stop=(kt == NKT - 1))
                        for wi, w_s in enumerate((wq_s, wk_s)):
                            for h in range(HPC):
                                msl = slice(h * P, (h + 1) * P)
                                nc.tensor.matmul(qk_ps[wi * HPC + h][:],
                                                 mm(w_s[:, kt, msl]),
                                                 mm(xt4[:, kk, :]), **fl)
                        for ts_ in range(nvp):
                            ssl = slice(ts_ * P, (ts_ + 1) * P)
                            nc.tensor.matmul(v_ps[ts_][:],
                                             mm(xt4[:, kk, ssl]),
                                             mm(wv_s[:, kt, :]), **fl)

                nc.gpsimd.dma_start(out=cos_t[:], in_=cost[:, tsl])
                nc.gpsimd.dma_start(out=sin_t[:], in_=sint[:, tsl])

                # rope eviction: dest = ps*cos + shuffle(ps)*sin_eff
                for wi, dest in ((0, qr), (1, kr)):
                    for h in range(HPC):
                        ps = qk_ps[wi * HPC + h]
                        shuf = rtmp_pool.tile([P, tchunk], F32, tag="shuf")
                        dst = dest[h][:, tsl]
                        nc.vector.stream_shuffle(out=shuf[:], in_=ps,
                                                 mask=SWAP_MASK)
                        nc.vector.tensor_mul(out=dst, in0=ps, in1=cos_t[:])
                        nc.vector.tensor_mul(out=shuf[:], in0=shuf[:], in1=sin_t[:])
                        nc.vector.tensor_add(out=dst, in0=dst, in1=shuf[:])

                # v eviction: token-major
                for ts_ in range(nvp):
                    nc.vector.tensor_copy(out=vv[:, c * nvp + ts_, :],
                                          in_=v_ps[ts_][:])

        nc.sync.dma_start(out=mask_s[:], in_=masks.rearrange("r p n -> p r n"))
        nc.sync.dma_start(out=wo_s[:],
                          in_=wot.rearrange("(mt p) o -> p mt o", p=P))

        # -------- phase 2+3: attention with interleaved output proj -------
        # Software-pipelined: QK for tile jt+1 issues before colsum/AV of jt,
        # and both heads' exp runs as ONE wide ACT op over a 2-bank PSUM
        # tile, so ACT latency never blocks the PE stream.
        with (
            tc.tile_pool(name="outp", bufs=1) as out_pool,
            tc.tile_pool(name="exps", bufs=8) as exps_pool,
            tc.tile_pool(name="rcp", bufs=2) as rcp_pool,
            tc.tile_pool(name="p2cs", bufs=1, space="PSUM") as p2cs,
            tc.tile_pool(name="p2av", bufs=1, space="PSUM") as p2av,
        ):
            outT = [out_pool.tile([P, t], FR, tag=f"outT{h}", name=f"outT{h}")
                    for h in range(HPC)]

            def drain_one(pend):
                (pes, plo, pw, pfl, pjt, ctx_) = pend.pop(0)
                (bb_, cs_l, av_l, isl_, c_) = ctx_
                for h in range(HPC):
                    nc.tensor.matmul(cs_l[h][:, plo:], mm(ones_s[:]),
                                     mm(pes[:, h, :pw]), **pfl)
                    nc.tensor.matmul(av_l[h][:, plo:],
                                     mm(vv[:, bb_ * (s // P) + pjt,
                                           h * P:(h + 1) * P]),
                                     mm(pes[:, h, :pw]), **pfl)
                if not pfl["stop"]:
                    return
                # chunk epilogue: normalize + output projection
                for h in range(HPC):
                    rcp = rcp_pool.tile([P, tchunk], F32, tag="rcp",
                                        name=f"rcp{h}_{bb_}_{c_}")
                    nc.vector.reciprocal_approx_fast(out=rcp[:], in_=cs_l[h][:])
                    nc.vector.tensor_mul(out=outT[h][:, isl_], in0=av_l[h][:],
                                         in1=rcp[:])
                wo_pools = [p2cs, p2cs, p2av, p2av]
                wo_tags = ["cs0", "cs1", "av0", "av1"]
                wi_ = 0
                for tt_ in range(tchunk // P):
                    tt0 = isl_.start + tt_ * P
                    ttsl = slice(tt0, tt0 + P)
                    for oc in range(H // 512):
                        osl = slice(oc * 512, (oc + 1) * 512)
                        ps = wo_pools[wi_ % 4].tile(
                            [P, 512], F32, tag=wo_tags[wi_ % 4],
                            name=f"wo_{tt0}_{oc}")
                        wi_ += 1
                        for h in range(HPC):
                            nc.tensor.matmul(ps[:],
                                             mm(outT[h][:, ttsl]),
                                             mm(wo_s[:, h, osl]),
                                             start=(h == 0),
                                             stop=(h == HPC - 1))
                        ev = ev_pool.tile([P, 512], FR, tag="ev",
                                          name=f"ev_{tt0}_{oc}")
                        nc.vector.tensor_copy(out=ev[:], in_=ps[:])
                        nc.sync.dma_start(out=out[ttsl, osl], in_=ev[:])

            pend = []
            for bb in range(b):
                for c in range(ich):
                    isl = slice(bb * s + c * tchunk, bb * s + (c + 1) * tchunk)
                    njt = r_mask * (c + 1)   # visible j-tiles
                    cs_ps = [p2cs.tile([P, tchunk], F32, tag=f"cs{h}",
                                       name=f"cs{h}_{bb}_{c}") for h in range(HPC)]
                    av_ps = [p2av.tile([P, tchunk], F32, tag=f"av{h}",
                                       name=f"av{h}_{bb}_{c}") for h in range(HPC)]
                    cctx = (bb, cs_ps, av_ps, isl, c)
                    for jt in range(njt):
                        jsl = slice(bb * s + jt * P, bb * s + (jt + 1) * P)
                        ri = jt - r_mask * c
                        lo = max(ri, 0) * P
                        w = tchunk - lo
                        csl = slice(isl.start + lo, isl.stop)
                        fl = dict(start=(jt == 0), stop=(jt == njt - 1))
                        sc = ab_pool.tile([P, HPC, 512], F32,
                                          tag=("A", "B")[jt % 2],
                                          name=f"sc_{bb}_{c}_{jt}")
                        for h in range(HPC):
                            nc.tensor.matmul(sc[:, h, :w], mm(kr[h][:, jsl]),
                                             mm(qr[h][:, csl]),
                                             start=True, stop=True)
                        es = exps_pool.tile([P, HPC, tchunk], FR, tag="es",
                                            name=f"es_{bb}_{c}_{jt}")
                        nc.scalar.activation(out=es[:, :, :w], in_=sc[:, :, :w],
                                             func=mybir.ActivationFunctionType.Exp,
                                             scale=float(SCALE))
                        if ri >= 0:  # diagonal-crossing tile
                            mb = mask_s[:, ri, lo:].unsqueeze(1).broadcast_to(
                                [P, HPC, w])
                            nc.vector.tensor_mul(out=es[:, :, :w],
                                                 in0=es[:, :, :w], in1=mb)
                        pend.append((es, lo, w, fl, jt, cctx))
                        if len(pend) > 2:
                            drain_one(pend)
            while pend:
                drain_one(pend)

    nc.compile()
    return nc


def make_in_maps(hidden_states, cos, sin, Wq, Wk, Wv, Wo, s=S, b=B, tchunk=512):
    t = b * s
    tchunk = min(tchunk, t)
    hs = np.asarray(hidden_states, np.float32).reshape(t, H)
    xt = np.ascontiguousarray(hs.T)
    cos2 = np.asarray(cos, np.float32).reshape(s, HD)
    sin2 = np.asarray(sin, np.float32).reshape(s, HD)
    cosP = np.ascontiguousarray(np.tile(cos2[:, PERM].T, (1, b)))
    sign = np.where(PERM < 64, -1.0, 1.0).astype(np.float32)[:, None]
    sinP = np.ascontiguousarray(np.tile(sin2[:, PERM].T * sign, (1, b)))
    masks_bf = build_masks(tchunk).astype(BF16)
    xt_bf = xt.astype(BF16)
    Wq, Wk, Wv, Wo = (np.asarray(w, np.float32) for w in (Wq, Wk, Wv, Wo))

    in_maps = []
    for c in range(NCORES):
        rows = np.concatenate([(HPC * c + hh) * HD + PERM for hh in range(HPC)])
        sl = slice(c * M, (c + 1) * M)
        in_maps.append({
            "xt": xt_bf,
            "wqt": np.ascontiguousarray(Wq[rows, :].T).astype(BF16),
            "wkt": np.ascontiguousarray(Wk[rows, :].T).astype(BF16),
            "wvt": np.ascontiguousarray(Wv[sl, :].T).astype(BF16),
            "wot": np.ascontiguousarray(Wo[:, sl].T).astype(BF16),
            "cost": cosP,
            "sint": sinP,
            "masks": masks_bf,
        })
    return in_maps


_CACHED_NC = None
_LAST_RESULTS = None


def kernel(hidden_states, cos, sin, Wq, Wk, Wv, Wo):
    global _CACHED_NC, _LAST_RESULTS
    in_maps = make_in_maps(hidden_states, cos, sin, Wq, Wk, Wv, Wo)
    if _CACHED_NC is None:
        _CACHED_NC = build_nc()
    res = run_bass_kernel_spmd(_CACHED_NC, in_maps, core_ids=list(range(NCORES)))
    _LAST_RESULTS = res
    acc = np.zeros((B * S, H), np.float32)
    for r in res.results:
        acc += r["out"].astype(np.float32)
    return acc.reshape(B, S, H)



# revision 5
# speedup vs baseline: 1.0577x; 1.0577x over previous
"""Trainium2 Bass kernel for nn_Attention_3126736192307.

Causal multi-head attention with RoPE: B=2, S=2048, H=2048, 16 heads x 128.

Sharding (tensor parallel over heads, 8 cores, 2 heads each):
  - Wq/Wk/Wv column-split (per-head), Wo row-split; each core computes a
    partial [B*S, H] output; the host sums the 8 partials (row-parallel
    unshard) - no on-device collectives needed.

Per-core dataflow (all matmuls transpose-free by construction):
  - Host pre-transposes: X.T [H, T], WqT/WkT [H, 256] (head-dim permuted so
    RoPE's rotate_half becomes an intra-quadrant stream_shuffle), WvT [H, 256],
    WoT [256, H], cos/sin [128, T] feature-major (sin sign-folded).
  - Phase 1: q,k feature-major [128, T] per head + RoPE (DVE); v token-major.
  - Phase 2 per (b, h, i-chunk): scores.T [j,i] = k.T (lhsT) @ q.T; exp on
    ScalarE (no max subtraction - scores are ~N(0,1) after the 1/sqrt(hd)
    scale); causal block skipping + 0/1 mask multiply on diagonal-crossing
    tiles; column sums via ones-matmul on TensorE; AV accumulation in PSUM;
    normalization folded into the PSUM->SBUF eviction.
  - Phase 3: out.T (lhsT) @ WoT -> partial [T, H], streamed to DRAM.

Matmuls run in bf16 (1 PE cycle/row; fp32 is 4x, and fp32r's fused
weight-load encoding can't carry the 2 semaphore waits Tile emits).

Stall-avoidance (trace-driven):
  - qr/kr are per-chunk tiles so phase-2 QK matmuls depend only on the
    chunk they read, not on the last chunk's RoPE eviction.
  - xt streams in half-chunk DMAs alternating sync/gpsimd queues with a
    4-deep buffer pool: each queue's DMA-semaphore reuse round-trip
    (issue -> complete -> reissue) stays off the critical path.
  - First k-tile of Wq/Wk/Wv arrives via 3 parallel queues so the first
    matmul starts ~6us earlier; cos/sin ride the otherwise-idle scalar
    queue.
  - A dummy exp at kernel start preloads the ACT table (1.3us).
  - Output DMAs round-robin sync/gpsimd/scalar so the final chunk's 2MB
    drains ~3x faster.
"""

import os
import sys

for _p in ("/opt/trn_rl_repo", "/root/.axon_site/_ro/trn_rl_repo"):
    if os.path.isdir(_p) and _p not in sys.path:
        sys.path.append(_p)

from contextlib import ExitStack

import ml_dtypes
import numpy as np

import concourse.bass as bass
import concourse.bacc as bacc
import concourse.tile as tile
from concourse import mybir
from concourse.bass_utils import run_bass_kernel_spmd

B, S, H, NH = 2, 2048, 2048, 16
HD = 128
NCORES = 8
HPC = NH // NCORES            # heads per core = 2
M = HPC * HD                  # 256 output channels per core
SCALE = HD ** -0.5
P = 128                       # partitions
NKT = H // P                  # 16 contraction tiles for projections

F32 = mybir.dt.float32

# head-dim permutation: interleave halves at 16 granularity so the RoPE
# partner (d <-> d+64) sits 16 partitions away inside one 32-part quadrant
PERM = np.concatenate([np.arange(16 * m, 16 * m + 16) + (64 if odd else 0)
                       for m in range(4) for odd in (0, 1)])
SWAP_MASK = [i ^ 16 for i in range(32)]


BF16 = ml_dtypes.bfloat16


def build_masks(tchunk):
    """0/1 keep-masks for the R diagonal-crossing j-tiles of each i-chunk."""
    r = tchunk // P
    m = np.zeros((r, P, tchunk), np.float32)
    il = np.arange(tchunk)
    for ri in range(r):
        for jl in range(P):
            m[ri, jl, :] = (P * ri + jl <= il).astype(np.float32)
    return m


def build_nc(s=S, b=B, tchunk=512, mm_dtype=mybir.dt.bfloat16):
    t = b * s
    tchunk = min(tchunk, t)
    nch = t // tchunk             # phase-1 token chunks
    ich = s // tchunk             # attention i-chunks per batch
    r_mask = tchunk // P          # diagonal-crossing tiles per i-chunk
    ntt = t // P                  # token tiles

    FR = mm_dtype

    def mm(ap):
        return ap

    nc = bacc.Bacc("TRN2", target_bir_lowering=False, debug=False)

    xt = nc.declare_dram_parameter("xt", [H, t], FR, isOutput=False)
    wqt = nc.declare_dram_parameter("wqt", [H, M], FR, isOutput=False)
    wkt = nc.declare_dram_parameter("wkt", [H, M], FR, isOutput=False)
    wvt = nc.declare_dram_parameter("wvt", [H, M], FR, isOutput=False)
    wot = nc.declare_dram_parameter("wot", [M, H], FR, isOutput=False)
    cost = nc.declare_dram_parameter("cost", [HD, t], F32, isOutput=False)
    sint = nc.declare_dram_parameter("sint", [HD, t], F32, isOutput=False)
    masks = nc.declare_dram_parameter("masks", [r_mask, P, tchunk], FR,
                                      isOutput=False)
    out = nc.declare_dram_parameter("out", [t, H], FR, isOutput=True)

    with tile.TileContext(nc) as tc, ExitStack() as ctx:
        persist = ctx.enter_context(tc.tile_pool(name="persist", bufs=1))

        # persistent activations; qr/kr are PER-CHUNK tiles so phase-2
        # reads only depend on the producing chunk's RoPE
        qr = [[persist.tile([P, tchunk], FR, tag=f"qr{h}_{c}", name=f"qr{h}_{c}")
               for c in range(nch)] for h in range(HPC)]
        kr = [[persist.tile([P, tchunk], FR, tag=f"kr{h}_{c}", name=f"kr{h}_{c}")
               for c in range(nch)] for h in range(HPC)]
        vv = persist.tile([P, ntt, M], FR, tag="vv")   # v[tt*128+p, d]
        ones_s = persist.tile([P, P], FR, tag="ones")
        nc.vector.memset(ones_s[:], 1.0)
        # preload the ACT exp table (1.3us) off the critical path
        warm_in = persist.tile([P, 2], F32, tag="warm_in")
        warm_out = persist.tile([P, 2], F32, tag="warm_out")
        nc.vector.memset(warm_in[:], 0.0)
        nc.scalar.activation(out=warm_out[:], in_=warm_in[:],
                             func=mybir.ActivationFunctionType.Exp,
                             scale=1.0)
        # allocated up-front (fresh SBUF -> no reuse waits on their DMAs);
        # loads issued after phase 1 so they don't delay the first matmuls
        mask_s = persist.tile([P, r_mask, tchunk], FR, tag="masks")
        wo_s = persist.tile([P, HPC, H], FR, tag="wo")
        ev_pool = ctx.enter_context(tc.tile_pool(name="evp", bufs=8))
        # whole-kernel 2-bank PSUM tiles: phase-1 q/k accumulator pairs and
        # attention score tiles rotate through the same two slots (A, B) --
        # no pool-handoff barrier on the critical QK path
        ab_pool = ctx.enter_context(tc.tile_pool(name="ab", bufs=1, space="PSUM"))

        # ---------------- phase 1: projections + rope -----------------
        with (
            tc.tile_pool(name="csin", bufs=2) as csin_pool,
            tc.tile_pool(name="xtp", bufs=4) as xt_pool,
            tc.tile_pool(name="rtmp", bufs=3) as rtmp_pool,
            tc.tile_pool(name="wts", bufs=1) as wts_pool,
            tc.tile_pool(name="p1v", bufs=1, space="PSUM") as p1v,
        ):
            wq_s = wts_pool.tile([P, NKT, M], FR, tag="wq")
            wk_s = wts_pool.tile([P, NKT, M], FR, tag="wk")
            wv_s = wts_pool.tile([P, NKT, M], FR, tag="wv")
            KG = 8                       # k-tiles per xt DMA (half chunk)
            for c in range(nch):
                tsl = slice(c * tchunk, (c + 1) * tchunk)
                cos_t = csin_pool.tile([P, tchunk], F32, tag="cos")
                sin_t = csin_pool.tile([P, tchunk], F32, tag="sin")

                # kt-outer: each X.T k-tile feeds all 8 accumulators, then dies
                q_ps = ab_pool.tile([P, HPC, 512], F32, tag="A", name=f"qps_{c}")
                k_ps = ab_pool.tile([P, HPC, 512], F32, tag="B", name=f"kps_{c}")
                qk_ps = [q_ps[:, 0, :tchunk], q_ps[:, 1, :tchunk],
                         k_ps[:, 0, :tchunk], k_ps[:, 1, :tchunk]]
                nvp = tchunk // P
                v_ps = [p1v.tile([P, M], F32, tag=f"p1v{i}",
                                 name=f"p1v{i}_{c}") for i in range(nvp)]
                if c == 0:
                    # first k-tile of each weight on its own queue (parallel
                    # issue); the rest stream in groups on gpsimd's SWDGE
                    for eng, w_s, wsrc in ((nc.sync, wq_s, wqt),
                                           (nc.scalar, wk_s, wkt),
                                           (nc.scalar, wv_s, wvt)):
                        eng.dma_start(
                            out=w_s[:, 0:1, :],
                            in_=wsrc[0:P, :].rearrange("(k p) m -> p k m", p=P))
                    for wg in range(NKT // 4):
                        wgsl = slice(max(wg * 4, 1), (wg + 1) * 4)
                        rsl = slice(wgsl.start * P, wgsl.stop * P)
                        for w_s, wsrc in ((wq_s, wqt), (wk_s, wkt),
                                          (wv_s, wvt)):
                            nc.gpsimd.dma_start(
                                out=w_s[:, wgsl, :],
                                in_=wsrc[rsl, :].rearrange(
                                    "(k p) m -> p k m", p=P))
                for g in range(NKT // KG):
                    gsl = slice(g * KG * P, (g + 1) * KG * P)
                    xt4 = xt_pool.tile([P, KG, tchunk], FR, tag="xt")
                    # c=0: gpsimd is busy issuing the 12 weight-group DMAs
                    xq = (nc.sync if c == 0 or (c * (NKT // KG) + g) % 2 == 0
                          else nc.gpsimd)
                    xq.dma_start(
                        out=xt4[:],
                        in_=xt[gsl, tsl].rearrange("(k p) t -> p k t", p=P))
                    for kk in range(KG):
                        kt = g * KG + kk
                        fl = dict(start=(kt == 0), stop=(kt == NKT - 1))
                        for wi, w_s in enumerate((wq_s, wk_s)):
                            for h in range(HPC):
                                msl = slice(h * P, (h + 1) * P)
                                nc.tensor.matmul(qk_ps[wi * HPC + h][:],
                                                 mm(w_s[:, kt, msl]),
                                                 mm(xt4[:, kk, :]), **fl)
                        for ts_ in range(nvp):
                            ssl = slice(ts_ * P, (ts_ + 1) * P)
                            nc.tensor.matmul(v_ps[ts_][:],
                                             mm(xt4[:, kk, ssl]),
                                             mm(wv_s[:, kt, :]), **fl)

                nc.scalar.dma_start(out=cos_t[:], in_=cost[:, tsl])
                nc.scalar.dma_start(out=sin_t[:], in_=sint[:, tsl])

                # rope eviction: dest = ps*cos + shuffle(ps)*sin_eff
                for wi, dest in ((0, qr), (1, kr)):
                    for h in range(HPC):
                        ps = qk_ps[wi * HPC + h]
                        shuf = rtmp_pool.tile([P, tchunk], F32, tag="shuf")
                        dst = dest[h][c][:, :]
                        nc.vector.stream_shuffle(out=shuf[:], in_=ps,
                                                 mask=SWAP_MASK)
                        nc.vector.tensor_mul(out=dst, in0=ps, in1=cos_t[:])
                        nc.vector.tensor_mul(out=shuf[:], in0=shuf[:], in1=sin_t[:])
                        nc.vector.tensor_add(out=dst, in0=dst, in1=shuf[:])

                # v eviction: token-major
                for ts_ in range(nvp):
                    nc.vector.tensor_copy(out=vv[:, c * nvp + ts_, :],
                                          in_=v_ps[ts_][:])

        nc.sync.dma_start(out=mask_s[:], in_=masks.rearrange("r p n -> p r n"))
        nc.sync.dma_start(out=wo_s[:],
                          in_=wot.rearrange("(mt p) o -> p mt o", p=P))

        # -------- phase 2+3: attention with interleaved output proj -------
        # Software-pipelined: QK for tile jt+1 issues before colsum/AV of jt,
        # and both heads' exp runs as ONE wide ACT op over a 2-bank PSUM
        # tile, so ACT latency never blocks the PE stream.
        with (
            tc.tile_pool(name="outp", bufs=1) as out_pool,
            tc.tile_pool(name="exps", bufs=8) as exps_pool,
            tc.tile_pool(name="rcp", bufs=2) as rcp_pool,
            tc.tile_pool(name="p2cs", bufs=1, space="PSUM") as p2cs,
            tc.tile_pool(name="p2av", bufs=1, space="PSUM") as p2av,
        ):
            outT = [out_pool.tile([P, t], FR, tag=f"outT{h}", name=f"outT{h}")
                    for h in range(HPC)]
            oqs = [nc.sync, nc.gpsimd, nc.scalar]   # out-DMA queue rotation

            def drain_one(pend):
                (pes, plo, pw, pfl, pjt, ctx_) = pend.pop(0)
                (bb_, cs_l, av_l, isl_, c_) = ctx_
                for h in range(HPC):
                    nc.tensor.matmul(cs_l[h][:, plo:], mm(ones_s[:]),
                                     mm(pes[:, h, :pw]), **pfl)
                    nc.tensor.matmul(av_l[h][:, plo:],
                                     mm(vv[:, bb_ * (s // P) + pjt,
                                           h * P:(h + 1) * P]),
                                     mm(pes[:, h, :pw]), **pfl)
                if not pfl["stop"]:
                    return
                # chunk epilogue: normalize + output projection
                for h in range(HPC):
                    rcp = rcp_pool.tile([P, tchunk], F32, tag="rcp",
                                        name=f"rcp{h}_{bb_}_{c_}")
                    nc.vector.reciprocal_approx_fast(out=rcp[:], in_=cs_l[h][:])
                    nc.vector.tensor_mul(out=outT[h][:, isl_], in0=av_l[h][:],
                                         in1=rcp[:])
                wo_pools = [p2cs, p2cs, p2av, p2av]
                wo_tags = ["cs0", "cs1", "av0", "av1"]
                wi_ = 0
                for tt_ in range(tchunk // P):
                    tt0 = isl_.start + tt_ * P
                    ttsl = slice(tt0, tt0 + P)
                    for oc in range(H // 512):
                        osl = slice(oc * 512, (oc + 1) * 512)
                        ps = wo_pools[wi_ % 4].tile(
                            [P, 512], F32, tag=wo_tags[wi_ % 4],
                            name=f"wo_{tt0}_{oc}")
                        for h in range(HPC):
                            nc.tensor.matmul(ps[:],
                                             mm(outT[h][:, ttsl]),
                                             mm(wo_s[:, h, osl]),
                                             start=(h == 0),
                                             stop=(h == HPC - 1))
                        ev = ev_pool.tile([P, 512], FR, tag="ev",
                                          name=f"ev_{tt0}_{oc}")
                        nc.vector.tensor_copy(out=ev[:], in_=ps[:])
                        oqs[wi_ % 3].dma_start(out=out[ttsl, osl], in_=ev[:])
                        wi_ += 1

            pend = []
            for bb in range(b):
                for c in range(ich):
                    isl = slice(bb * s + c * tchunk, bb * s + (c + 1) * tchunk)
                    njt = r_mask * (c + 1)   # visible j-tiles
                    cs_ps = [p2cs.tile([P, tchunk], F32, tag=f"cs{h}",
                                       name=f"cs{h}_{bb}_{c}") for h in range(HPC)]
                    av_ps = [p2av.tile([P, tchunk], F32, tag=f"av{h}",
                                       name=f"av{h}_{bb}_{c}") for h in range(HPC)]
                    cctx = (bb, cs_ps, av_ps, isl, c)
                    for jt in range(njt):
                        jc = bb * ich + jt // r_mask
                        jlo = (jt % r_mask) * P
                        ri = jt - r_mask * c
                        lo = max(ri, 0) * P
                        w = tchunk - lo
                        fl = dict(start=(jt == 0), stop=(jt == njt - 1))
                        sc = ab_pool.tile([P, HPC, 512], F32,
                                          tag=("A", "B")[jt % 2],
                                          name=f"sc_{bb}_{c}_{jt}")
                        for h in range(HPC):
                            nc.tensor.matmul(
                                sc[:, h, :w],
                                mm(kr[h][jc][:, jlo:jlo + P]),
                                mm(qr[h][bb * ich + c][:, lo:]),
                                start=True, stop=True)
                        es = exps_pool.tile([P, HPC, tchunk], FR, tag="es",
                                            name=f"es_{bb}_{c}_{jt}")
                        nc.scalar.activation(out=es[:, :, :w], in_=sc[:, :, :w],
                                             func=mybir.ActivationFunctionType.Exp,
                                             scale=float(SCALE))
                        if ri >= 0:  # diagonal-crossing tile
                            mb = mask_s[:, ri, lo:].unsqueeze(1).broadcast_to(
                                [P, HPC, w])
                            nc.vector.tensor_mul(out=es[:, :, :w],
                                                 in0=es[:, :, :w], in1=mb)
                        pend.append((es, lo, w, fl, jt, cctx))
                        if len(pend) > 2:
                            drain_one(pend)
            while pend:
                drain_one(pend)

    nc.compile()
    return nc


def make_in_maps(hidden_states, cos, sin, Wq, Wk, Wv, Wo, s=S, b=B, tchunk=512):
    t = b * s
    tchunk = min(tchunk, t)
    hs = np.asarray(hidden_states, np.float32).reshape(t, H)
    xt = np.ascontiguousarray(hs.T)
    cos2 = np.asarray(cos, np.float32).reshape(s, HD)
    sin2 = np.asarray(sin, np.float32).reshape(s, HD)
    cosP = np.ascontiguousarray(np.tile(cos2[:, PERM].T, (1, b)))
    sign = np.where(PERM < 64, -1.0, 1.0).astype(np.float32)[:, None]
    sinP = np.ascontiguousarray(np.tile(sin2[:, PERM].T * sign, (1, b)))
    masks_bf = build_masks(tchunk).astype(BF16)
    xt_bf = xt.astype(BF16)
    Wq, Wk, Wv, Wo = (np.asarray(w, np.float32) for w in (Wq, Wk, Wv, Wo))

    in_maps = []
    for c in range(NCORES):
        rows = np.concatenate([(HPC * c + hh) * HD + PERM for hh in range(HPC)])
        sl = slice(c * M, (c + 1) * M)
        in_maps.append({
            "xt": xt_bf,
            "wqt": np.ascontiguousarray(Wq[rows, :].T).astype(BF16),
            "wkt": np.ascontiguousarray(Wk[rows, :].T).astype(BF16),
            "wvt": np.ascontiguousarray(Wv[sl, :].T).astype(BF16),
            "wot": np.ascontiguousarray(Wo[:, sl].T).astype(BF16),
            "cost": cosP,
            "sint": sinP,
            "masks": masks_bf,
        })
    return in_maps


_CACHED_NC = None
_LAST_RESULTS = None


def kernel(hidden_states, cos, sin, Wq, Wk, Wv, Wo):
    global _CACHED_NC, _LAST_RESULTS
    in_maps = make_in_maps(hidden_states, cos, sin, Wq, Wk, Wv, Wo)
    if _CACHED_NC is None:
        _CACHED_NC = build_nc()
    res = run_bass_kernel_spmd(_CACHED_NC, in_maps, core_ids=list(range(NCORES)))
    _LAST_RESULTS = res
    acc = np.zeros((B * S, H), np.float32)
    for r in res.results:
        acc += r["out"].astype(np.float32)
    return acc.reshape(B, S, H)


# revision 9
# speedup vs baseline: 1.0858x; 1.0266x over previous
"""Trainium2 Bass kernel for nn_Attention_3126736192307.

Causal multi-head attention with RoPE: B=2, S=2048, H=2048, 16 heads x 128.

Sharding (tensor parallel over heads, 8 cores, 2 heads each):
  - Wq/Wk/Wv column-split (per-head), Wo row-split; each core computes a
    partial [B*S, H] output; the host sums the 8 partials (row-parallel
    unshard) - no on-device collectives needed.

Per-core dataflow (all matmuls transpose-free by construction):
  - Host pre-transposes: X.T [H, T], WqT/WkT [H, 256] (head-dim permuted so
    RoPE's rotate_half becomes an intra-quadrant stream_shuffle), WvT [H, 256],
    WoT [256, H], cos/sin [128, T] feature-major (sin sign-folded).
  - Phase 1: q,k feature-major [128, T] per head + RoPE (DVE); v token-major.
  - Phase 2 per (b, h, i-chunk): scores.T [j,i] = k.T (lhsT) @ q.T; exp on
    ScalarE (no max subtraction - scores are ~N(0,1) after the 1/sqrt(hd)
    scale); causal block skipping + 0/1 mask multiply on diagonal-crossing
    tiles; column sums via ones-matmul on TensorE; AV accumulation in PSUM;
    normalization folded into the PSUM->SBUF eviction.
  - Phase 3: out.T (lhsT) @ WoT -> partial [T, H], streamed to DRAM.

Matmuls run in bf16 (1 PE cycle/row; fp32 is 4x, and fp32r's fused
weight-load encoding can't carry the 2 semaphore waits Tile emits).

Stall-avoidance (trace-driven):
  - qr/kr are per-chunk tiles so phase-2 QK matmuls depend only on the
    chunk they read, not on the last chunk's RoPE eviction.
  - xt streams in half-chunk DMAs alternating sync/gpsimd queues with a
    4-deep buffer pool: each queue's DMA-semaphore reuse round-trip
    (issue -> complete -> reissue) stays off the critical path.
  - First k-tile of Wq/Wk/Wv arrives via 3 parallel queues so the first
    matmul starts ~6us earlier; cos/sin ride the otherwise-idle scalar
    queue.
  - A dummy exp at kernel start preloads the ACT table (1.3us).
  - Output DMAs round-robin sync/gpsimd/scalar so the final chunk's 2MB
    drains ~3x faster.
"""

import os
import sys

for _p in ("/opt/trn_rl_repo", "/root/.axon_site/_ro/trn_rl_repo"):
    if os.path.isdir(_p) and _p not in sys.path:
        sys.path.append(_p)

from contextlib import ExitStack

import ml_dtypes
import numpy as np

import concourse.bass as bass
import concourse.bacc as bacc
import concourse.tile as tile
from concourse import mybir
from concourse.bass_utils import run_bass_kernel_spmd

B, S, H, NH = 2, 2048, 2048, 16
HD = 128
NCORES = 8
HPC = NH // NCORES            # heads per core = 2
M = HPC * HD                  # 256 output channels per core
SCALE = HD ** -0.5
P = 128                       # partitions
NKT = H // P                  # 16 contraction tiles for projections

F32 = mybir.dt.float32

# head-dim permutation: interleave halves at 16 granularity so the RoPE
# partner (d <-> d+64) sits 16 partitions away inside one 32-part quadrant
PERM = np.concatenate([np.arange(16 * m, 16 * m + 16) + (64 if odd else 0)
                       for m in range(4) for odd in (0, 1)])
SWAP_MASK = [i ^ 16 for i in range(32)]


BF16 = ml_dtypes.bfloat16


def build_masks(tchunk):
    """0/1 keep-masks for the R diagonal-crossing j-tiles of each i-chunk."""
    r = tchunk // P
    m = np.zeros((r, P, tchunk), np.float32)
    il = np.arange(tchunk)
    for ri in range(r):
        for jl in range(P):
            m[ri, jl, :] = (P * ri + jl <= il).astype(np.float32)
    return m


def build_nc(s=S, b=B, tchunk=512, mm_dtype=mybir.dt.bfloat16):
    t = b * s
    tchunk = min(tchunk, t)
    nch = t // tchunk             # phase-1 token chunks
    ich = s // tchunk             # attention i-chunks per batch
    r_mask = tchunk // P          # diagonal-crossing tiles per i-chunk
    ntt = t // P                  # token tiles

    FR = mm_dtype

    def mm(ap):
        return ap

    nc = bacc.Bacc("TRN2", target_bir_lowering=False, debug=False)

    xt = nc.declare_dram_parameter("xt", [H, t], FR, isOutput=False)
    wqt = nc.declare_dram_parameter("wqt", [H, M], FR, isOutput=False)
    wkt = nc.declare_dram_parameter("wkt", [H, M], FR, isOutput=False)
    wvt = nc.declare_dram_parameter("wvt", [H, M], FR, isOutput=False)
    wot = nc.declare_dram_parameter("wot", [M, H], FR, isOutput=False)
    cost = nc.declare_dram_parameter("cost", [HD, t], F32, isOutput=False)
    sint = nc.declare_dram_parameter("sint", [HD, t], F32, isOutput=False)
    masks = nc.declare_dram_parameter("masks", [r_mask, P, tchunk], FR,
                                      isOutput=False)
    out = nc.declare_dram_parameter("out", [t, H], FR, isOutput=True)

    with tile.TileContext(nc) as tc, ExitStack() as ctx:
        persist = ctx.enter_context(tc.tile_pool(name="persist", bufs=1))

        # persistent activations; qr/kr are PER-CHUNK tiles so phase-2
        # reads only depend on the producing chunk's RoPE
        qr = [[persist.tile([P, tchunk], FR, tag=f"qr{h}_{c}", name=f"qr{h}_{c}")
               for c in range(nch)] for h in range(HPC)]
        kr = [[persist.tile([P, tchunk], FR, tag=f"kr{h}_{c}", name=f"kr{h}_{c}")
               for c in range(nch)] for h in range(HPC)]
        vv = persist.tile([P, ntt, M], FR, tag="vv")   # v[tt*128+p, d]
        ones_s = persist.tile([P, P], FR, tag="ones")
        nc.vector.memset(ones_s[:], 1.0)
        # preload the ACT exp table (1.3us) off the critical path
        warm_in = persist.tile([P, 2], F32, tag="warm_in")
        warm_out = persist.tile([P, 2], F32, tag="warm_out")
        nc.vector.memset(warm_in[:], 0.0)
        nc.scalar.activation(out=warm_out[:], in_=warm_in[:],
                             func=mybir.ActivationFunctionType.Exp,
                             scale=1.0)
        # allocated up-front (fresh SBUF -> no reuse waits on their DMAs);
        # loads issued after phase 1 so they don't delay the first matmuls
        mask_s = persist.tile([P, r_mask, tchunk], FR, tag="masks")
        wo_s = persist.tile([P, HPC, H], FR, tag="wo")
        ev_pool = ctx.enter_context(tc.tile_pool(name="evp", bufs=16))
        # whole-kernel 2-bank PSUM tiles: phase-1 q/k accumulator pairs and
        # attention score tiles rotate through the same two slots (A, B) --
        # no pool-handoff barrier on the critical QK path
        ab_pool = ctx.enter_context(tc.tile_pool(name="ab", bufs=1, space="PSUM"))

        # ---------------- phase 1: projections + rope -----------------
        with (
            tc.tile_pool(name="csin", bufs=2) as csin_pool,
            tc.tile_pool(name="xtp", bufs=8) as xt_pool,
            tc.tile_pool(name="rtmp", bufs=3) as rtmp_pool,
            tc.tile_pool(name="wts", bufs=1) as wts_pool,
            tc.tile_pool(name="p1v", bufs=1, space="PSUM") as p1v,
        ):
            wq_s = wts_pool.tile([P, NKT, M], FR, tag="wq")
            wk_s = wts_pool.tile([P, NKT, M], FR, tag="wk")
            wv_s = wts_pool.tile([P, NKT, M], FR, tag="wv")
            KG = 4                       # k-tiles per xt DMA
            # round-robin DMA queues; chunk 0 issues weights and xt in
            # k-tile order so arrivals track the compute order (all queues
            # share ~400GB/s aggregate -- issue order IS arrival order)
            rr = [nc.sync, nc.gpsimd, nc.scalar]
            rri = 0
            for c in range(nch):
                tsl = slice(c * tchunk, (c + 1) * tchunk)
                cos_t = csin_pool.tile([P, tchunk], F32, tag="cos")
                sin_t = csin_pool.tile([P, tchunk], F32, tag="sin")

                # kt-outer: each X.T k-tile feeds all 8 accumulators, then dies
                q_ps = ab_pool.tile([P, HPC, 512], F32, tag="A", name=f"qps_{c}")
                k_ps = ab_pool.tile([P, HPC, 512], F32, tag="B", name=f"kps_{c}")
                qk_ps = [q_ps[:, 0, :tchunk], q_ps[:, 1, :tchunk],
                         k_ps[:, 0, :tchunk], k_ps[:, 1, :tchunk]]
                nvp = tchunk // P
                v_ps = [p1v.tile([P, M], F32, tag=f"p1v{i}",
                                 name=f"p1v{i}_{c}") for i in range(nvp)]
                for g in range(NKT // KG):
                    gsl = slice(g * KG * P, (g + 1) * KG * P)
                    if c == 0:
                        for w_s, wsrc in ((wq_s, wqt), (wk_s, wkt),
                                          (wv_s, wvt)):
                            rr[rri % 3].dma_start(
                                out=w_s[:, g * KG:(g + 1) * KG, :],
                                in_=wsrc[gsl, :].rearrange(
                                    "(k p) m -> p k m", p=P))
                            rri += 1
                    xt4 = xt_pool.tile([P, KG, tchunk], FR, tag="xt")
                    if c == 0:
                        xq = rr[rri % 3]
                        rri += 1
                    else:
                        xq = rr[g % 2]   # sync/gpsimd; scalar has cos/sin
                    xq.dma_start(
                        out=xt4[:],
                        in_=xt[gsl, tsl].rearrange("(k p) t -> p k t", p=P))
                    for kk in range(KG):
                        kt = g * KG + kk
                        fl = dict(start=(kt == 0), stop=(kt == NKT - 1))
                        for wi, w_s in enumerate((wq_s, wk_s)):
                            for h in range(HPC):
                                msl = slice(h * P, (h + 1) * P)
                                nc.tensor.matmul(qk_ps[wi * HPC + h][:],
                                                 mm(w_s[:, kt, msl]),
                                                 mm(xt4[:, kk, :]), **fl)
                        for ts_ in range(nvp):
                            ssl = slice(ts_ * P, (ts_ + 1) * P)
                            nc.tensor.matmul(v_ps[ts_][:],
                                             mm(xt4[:, kk, ssl]),
                                             mm(wv_s[:, kt, :]), **fl)

                nc.scalar.dma_start(out=cos_t[:], in_=cost[:, tsl])
                nc.scalar.dma_start(out=sin_t[:], in_=sint[:, tsl])
                if c == 1:
                    # phase-2 constants: after the startup burst, long
                    # before first use (~200us)
                    nc.scalar.dma_start(out=mask_s[:],
                                        in_=masks.rearrange("r p n -> p r n"))
                    nc.scalar.dma_start(
                        out=wo_s[:],
                        in_=wot.rearrange("(mt p) o -> p mt o", p=P))

                # rope eviction: dest = ps*cos + shuffle(ps)*sin_eff
                for wi, dest in ((0, qr), (1, kr)):
                    for h in range(HPC):
                        ps = qk_ps[wi * HPC + h]
                        shuf = rtmp_pool.tile([P, tchunk], F32, tag="shuf")
                        dst = dest[h][c][:, :]
                        nc.vector.stream_shuffle(out=shuf[:], in_=ps,
                                                 mask=SWAP_MASK)
                        nc.vector.tensor_mul(out=dst, in0=ps, in1=cos_t[:])
                        nc.vector.tensor_mul(out=shuf[:], in0=shuf[:], in1=sin_t[:])
                        nc.vector.tensor_add(out=dst, in0=dst, in1=shuf[:])

                # v eviction: token-major
                for ts_ in range(nvp):
                    nc.vector.tensor_copy(out=vv[:, c * nvp + ts_, :],
                                          in_=v_ps[ts_][:])

        # -------- phase 2+3: attention with interleaved output proj -------
        # Software-pipelined: QK for tile jt+1 issues before colsum/AV of jt,
        # and both heads' exp runs as ONE wide ACT op over a 2-bank PSUM
        # tile, so ACT latency never blocks the PE stream.
        with (
            tc.tile_pool(name="outp", bufs=1) as out_pool,
            tc.tile_pool(name="exps", bufs=8) as exps_pool,
            tc.tile_pool(name="rcp", bufs=2) as rcp_pool,
            tc.tile_pool(name="p2cs", bufs=1, space="PSUM") as p2cs,
            tc.tile_pool(name="p2av", bufs=1, space="PSUM") as p2av,
        ):
            outT = [out_pool.tile([P, t], FR, tag=f"outT{h}", name=f"outT{h}")
                    for h in range(HPC)]
            oqs = [nc.sync, nc.gpsimd, nc.scalar]   # out-DMA queue rotation

            def drain_one(pend):
                (pes, plo, pw, pfl, pjt, ctx_) = pend.pop(0)
                (bb_, cs_l, av_l, isl_, c_) = ctx_
                for h in range(HPC):
                    nc.tensor.matmul(cs_l[h][:, plo:], mm(ones_s[:]),
                                     mm(pes[:, h, :pw]), **pfl)
                    nc.tensor.matmul(av_l[h][:, plo:],
                                     mm(vv[:, bb_ * (s // P) + pjt,
                                           h * P:(h + 1) * P]),
                                     mm(pes[:, h, :pw]), **pfl)
                if not pfl["stop"]:
                    return
                # chunk epilogue: normalize + output projection
                for h in range(HPC):
                    rcp = rcp_pool.tile([P, tchunk], F32, tag="rcp",
                                        name=f"rcp{h}_{bb_}_{c_}")
                    nc.vector.reciprocal_approx_fast(out=rcp[:], in_=cs_l[h][:])
                    nc.vector.tensor_mul(out=outT[h][:, isl_], in0=av_l[h][:],
                                         in1=rcp[:])
                wo_pools = [p2cs, p2cs, p2av, p2av]
                wo_tags = ["cs0", "cs1", "av0", "av1"]
                wi_ = 0
                for tt_ in range(tchunk // P):
                    tt0 = isl_.start + tt_ * P
                    ttsl = slice(tt0, tt0 + P)
                    for oc in range(H // 512):
                        osl = slice(oc * 512, (oc + 1) * 512)
                        ps = wo_pools[wi_ % 4].tile(
                            [P, 512], F32, tag=wo_tags[wi_ % 4],
                            name=f"wo_{tt0}_{oc}")
                        for h in range(HPC):
                            nc.tensor.matmul(ps[:],
                                             mm(outT[h][:, ttsl]),
                                             mm(wo_s[:, h, osl]),
                                             start=(h == 0),
                                             stop=(h == HPC - 1))
                        ev = ev_pool.tile([P, 512], FR, tag="ev",
                                          name=f"ev_{tt0}_{oc}")
                        nc.vector.tensor_copy(out=ev[:], in_=ps[:])
                        oqs[wi_ % 3].dma_start(out=out[ttsl, osl], in_=ev[:])
                        wi_ += 1

            pend = []
            for bb in range(b):
                for c in range(ich):
                    isl = slice(bb * s + c * tchunk, bb * s + (c + 1) * tchunk)
                    njt = r_mask * (c + 1)   # visible j-tiles
                    cs_ps = [p2cs.tile([P, tchunk], F32, tag=f"cs{h}",
                                       name=f"cs{h}_{bb}_{c}") for h in range(HPC)]
                    av_ps = [p2av.tile([P, tchunk], F32, tag=f"av{h}",
                                       name=f"av{h}_{bb}_{c}") for h in range(HPC)]
                    cctx = (bb, cs_ps, av_ps, isl, c)
                    for jt in range(njt):
                        jc = bb * ich + jt // r_mask
                        jlo = (jt % r_mask) * P
                        ri = jt - r_mask * c
                        lo = max(ri, 0) * P
                        w = tchunk - lo
                        fl = dict(start=(jt == 0), stop=(jt == njt - 1))
                        sc = ab_pool.tile([P, HPC, 512], F32,
                                          tag=("A", "B")[jt % 2],
                                          name=f"sc_{bb}_{c}_{jt}")
                        for h in range(HPC):
                            nc.tensor.matmul(
                                sc[:, h, :w],
                                mm(kr[h][jc][:, jlo:jlo + P]),
                                mm(qr[h][bb * ich + c][:, lo:]),
                                start=True, stop=True)
                        es = exps_pool.tile([P, HPC, tchunk], FR, tag="es",
                                            name=f"es_{bb}_{c}_{jt}")
                        nc.scalar.activation(out=es[:, :, :w], in_=sc[:, :, :w],
                                             func=mybir.ActivationFunctionType.Exp,
                                             scale=float(SCALE))
                        if ri >= 0:  # diagonal-crossing tile
                            mb = mask_s[:, ri, lo:].unsqueeze(1).broadcast_to(
                                [P, HPC, w])
                            nc.vector.tensor_mul(out=es[:, :, :w],
                                                 in0=es[:, :, :w], in1=mb)
                        pend.append((es, lo, w, fl, jt, cctx))
                        if len(pend) > 2:
                            drain_one(pend)
            while pend:
                drain_one(pend)

    nc.compile()
    return nc


def make_in_maps(hidden_states, cos, sin, Wq, Wk, Wv, Wo, s=S, b=B, tchunk=512):
    t = b * s
    tchunk = min(tchunk, t)
    hs = np.asarray(hidden_states, np.float32).reshape(t, H)
    xt = np.ascontiguousarray(hs.T)
    cos2 = np.asarray(cos, np.float32).reshape(s, HD)
    sin2 = np.asarray(sin, np.float32).reshape(s, HD)
    cosP = np.ascontiguousarray(np.tile(cos2[:, PERM].T, (1, b)))
    sign = np.where(PERM < 64, -1.0, 1.0).astype(np.float32)[:, None]
    sinP = np.ascontiguousarray(np.tile(sin2[:, PERM].T * sign, (1, b)))
    masks_bf = build_masks(tchunk).astype(BF16)
    xt_bf = xt.astype(BF16)
    Wq, Wk, Wv, Wo = (np.asarray(w, np.float32) for w in (Wq, Wk, Wv, Wo))

    in_maps = []
    for c in range(NCORES):
        rows = np.concatenate([(HPC * c + hh) * HD + PERM for hh in range(HPC)])
        sl = slice(c * M, (c + 1) * M)
        in_maps.append({
            "xt": xt_bf,
            "wqt": np.ascontiguousarray(Wq[rows, :].T).astype(BF16),
            "wkt": np.ascontiguousarray(Wk[rows, :].T).astype(BF16),
            "wvt": np.ascontiguousarray(Wv[sl, :].T).astype(BF16),
            "wot": np.ascontiguousarray(Wo[:, sl].T).astype(BF16),
            "cost": cosP,
            "sint": sinP,
            "masks": masks_bf,
        })
    return in_maps


_CACHED_NC = None
_LAST_RESULTS = None


def kernel(hidden_states, cos, sin, Wq, Wk, Wv, Wo):
    global _CACHED_NC, _LAST_RESULTS
    in_maps = make_in_maps(hidden_states, cos, sin, Wq, Wk, Wv, Wo)
    if _CACHED_NC is None:
        _CACHED_NC = build_nc()
    res = run_bass_kernel_spmd(_CACHED_NC, in_maps, core_ids=list(range(NCORES)))
    _LAST_RESULTS = res
    acc = np.zeros((B * S, H), np.float32)
    for r in res.results:
        acc += r["out"].astype(np.float32)
    return acc.reshape(B, S, H)


# revision 14
# speedup vs baseline: 1.0894x; 1.0032x over previous
"""Trainium2 Bass kernel for nn_Attention_3126736192307.

Causal multi-head attention with RoPE: B=2, S=2048, H=2048, 16 heads x 128.

Sharding (tensor parallel over heads, 8 cores, 2 heads each):
  - Wq/Wk/Wv column-split (per-head), Wo row-split; each core computes a
    partial [B*S, H] output; the host sums the 8 partials (row-parallel
    unshard) - no on-device collectives needed.

Per-core dataflow (all matmuls transpose-free by construction):
  - Host pre-transposes: X.T [H, T], WqT/WkT [H, 256] (head-dim permuted so
    RoPE's rotate_half becomes an intra-quadrant stream_shuffle), WvT [H, 256],
    WoT [256, H], cos/sin [128, T] feature-major (sin sign-folded).
  - Phase 1: q,k feature-major [128, T] per head + RoPE (DVE); v token-major.
  - Phase 2 per (b, h, i-chunk): scores.T [j,i] = k.T (lhsT) @ q.T; exp on
    ScalarE (no max subtraction - scores are ~N(0,1) after the 1/sqrt(hd)
    scale); causal block skipping + 0/1 mask multiply on diagonal-crossing
    tiles; column sums via ones-matmul on TensorE; AV accumulation in PSUM;
    normalization folded into the PSUM->SBUF eviction.
  - Phase 3: out.T (lhsT) @ WoT -> partial [T, H], streamed to DRAM.

Matmuls run in bf16 (1 PE cycle/row; fp32 is 4x, and fp32r's fused
weight-load encoding can't carry the 2 semaphore waits Tile emits).

Stall-avoidance (trace-driven):
  - qr/kr are per-chunk tiles so phase-2 QK matmuls depend only on the
    chunk they read, not on the last chunk's RoPE eviction.
  - xt streams in half-chunk DMAs alternating sync/gpsimd queues with a
    4-deep buffer pool: each queue's DMA-semaphore reuse round-trip
    (issue -> complete -> reissue) stays off the critical path.
  - First k-tile of Wq/Wk/Wv arrives via 3 parallel queues so the first
    matmul starts ~6us earlier; cos/sin ride the otherwise-idle scalar
    queue.
  - A dummy exp at kernel start preloads the ACT table (1.3us).
  - Output DMAs round-robin sync/gpsimd/scalar so the final chunk's 2MB
    drains ~3x faster.
"""

import os
import sys

for _p in ("/opt/trn_rl_repo", "/root/.axon_site/_ro/trn_rl_repo"):
    if os.path.isdir(_p) and _p not in sys.path:
        sys.path.append(_p)

from contextlib import ExitStack

import ml_dtypes
import numpy as np

import concourse.bass as bass
import concourse.bacc as bacc
import concourse.tile as tile
from concourse import mybir
from concourse.bass_utils import run_bass_kernel_spmd

B, S, H, NH = 2, 2048, 2048, 16
HD = 128
NCORES = 8
HPC = NH // NCORES            # heads per core = 2
M = HPC * HD                  # 256 output channels per core
SCALE = HD ** -0.5
P = 128                       # partitions
NKT = H // P                  # 16 contraction tiles for projections

F32 = mybir.dt.float32

# head-dim permutation: interleave halves at 16 granularity so the RoPE
# partner (d <-> d+64) sits 16 partitions away inside one 32-part quadrant
PERM = np.concatenate([np.arange(16 * m, 16 * m + 16) + (64 if odd else 0)
                       for m in range(4) for odd in (0, 1)])
SWAP_MASK = [i ^ 16 for i in range(32)]


BF16 = ml_dtypes.bfloat16


def build_masks(tchunk):
    """0/1 keep-masks for the R diagonal-crossing j-tiles of each i-chunk."""
    r = tchunk // P
    m = np.zeros((r, P, tchunk), np.float32)
    il = np.arange(tchunk)
    for ri in range(r):
        for jl in range(P):
            m[ri, jl, :] = (P * ri + jl <= il).astype(np.float32)
    return m


def build_nc(s=S, b=B, tchunk=512, mm_dtype=mybir.dt.bfloat16):
    t = b * s
    tchunk = min(tchunk, t)
    nch = t // tchunk             # phase-1 token chunks
    ich = s // tchunk             # attention i-chunks per batch
    r_mask = tchunk // P          # diagonal-crossing tiles per i-chunk
    ntt = t // P                  # token tiles

    FR = mm_dtype

    def mm(ap):
        return ap

    nc = bacc.Bacc("TRN2", target_bir_lowering=False, debug=False)

    xt = nc.declare_dram_parameter("xt", [H, t], FR, isOutput=False)
    wqt = nc.declare_dram_parameter("wqt", [H, M], FR, isOutput=False)
    wkt = nc.declare_dram_parameter("wkt", [H, M], FR, isOutput=False)
    wvt = nc.declare_dram_parameter("wvt", [H, M], FR, isOutput=False)
    wot = nc.declare_dram_parameter("wot", [M, H], FR, isOutput=False)
    cost = nc.declare_dram_parameter("cost", [HD, t], F32, isOutput=False)
    sint = nc.declare_dram_parameter("sint", [HD, t], F32, isOutput=False)
    masks = nc.declare_dram_parameter("masks", [r_mask, P, tchunk], FR,
                                      isOutput=False)
    out = nc.declare_dram_parameter("out", [t, H], FR, isOutput=True)

    with tile.TileContext(nc) as tc, ExitStack() as ctx:
        persist = ctx.enter_context(tc.tile_pool(name="persist", bufs=1))

        # persistent activations; qr/kr are PER-CHUNK tiles so phase-2
        # reads only depend on the producing chunk's RoPE
        qr = [[persist.tile([P, tchunk], FR, tag=f"qr{h}_{c}", name=f"qr{h}_{c}")
               for c in range(nch)] for h in range(HPC)]
        kr = [[persist.tile([P, tchunk], FR, tag=f"kr{h}_{c}", name=f"kr{h}_{c}")
               for c in range(nch)] for h in range(HPC)]
        vv = persist.tile([P, ntt, M], FR, tag="vv")   # v[tt*128+p, d]
        ones_s = persist.tile([P, P], FR, tag="ones")
        nc.vector.memset(ones_s[:], 1.0)
        # preload the ACT exp table (1.3us) off the critical path
        warm_in = persist.tile([P, 2], F32, tag="warm_in")
        warm_out = persist.tile([P, 2], F32, tag="warm_out")
        nc.vector.memset(warm_in[:], 0.0)
        nc.scalar.activation(out=warm_out[:], in_=warm_in[:],
                             func=mybir.ActivationFunctionType.Exp,
                             scale=1.0)
        # allocated up-front (fresh SBUF -> no reuse waits on their DMAs);
        # loads issued after phase 1 so they don't delay the first matmuls
        mask_s = persist.tile([P, r_mask, tchunk], FR, tag="masks")
        wo_s = persist.tile([P, HPC, H], FR, tag="wo")
        ev_pool = ctx.enter_context(tc.tile_pool(name="evp", bufs=6))
        # whole-kernel 2-bank PSUM tiles: phase-1 q/k accumulator pairs and
        # attention score tiles rotate through the same two slots (A, B) --
        # no pool-handoff barrier on the critical QK path
        ab_pool = ctx.enter_context(tc.tile_pool(name="ab", bufs=1, space="PSUM"))

        # ---------------- phase 1: projections + rope -----------------
        with (
            tc.tile_pool(name="csin", bufs=2) as csin_pool,
            tc.tile_pool(name="xtp", bufs=8) as xt_pool,
            tc.tile_pool(name="rtmp", bufs=3) as rtmp_pool,
            tc.tile_pool(name="qkc", bufs=3) as qkc_pool,
            tc.tile_pool(name="wts", bufs=1) as wts_pool,
            tc.tile_pool(name="p1v", bufs=1, space="PSUM") as p1v,
        ):
            wq_s = wts_pool.tile([P, NKT, M], FR, tag="wq")
            wk_s = wts_pool.tile([P, NKT, M], FR, tag="wk")
            wv_s = wts_pool.tile([P, NKT, M], FR, tag="wv")
            KG = 4                       # k-tiles per xt DMA
            # round-robin DMA queues; chunk 0 issues weights and xt in
            # k-tile order so arrivals track the compute order (all queues
            # share ~400GB/s aggregate -- issue order IS arrival order)
            rr = [nc.sync, nc.gpsimd, nc.scalar]
            rri = 0
            for c in range(nch):
                tsl = slice(c * tchunk, (c + 1) * tchunk)
                cos_t = csin_pool.tile([P, tchunk], F32, tag="cos")
                sin_t = csin_pool.tile([P, tchunk], F32, tag="sin")

                # kt-outer: each X.T k-tile feeds all 8 accumulators, then dies
                q_ps = ab_pool.tile([P, HPC, 512], F32, tag="A", name=f"qps_{c}")
                k_ps = ab_pool.tile([P, HPC, 512], F32, tag="B", name=f"kps_{c}")
                qk_ps = [q_ps[:, 0, :tchunk], q_ps[:, 1, :tchunk],
                         k_ps[:, 0, :tchunk], k_ps[:, 1, :tchunk]]
                nvp = tchunk // P
                v_ps = [p1v.tile([P, M], F32, tag=f"p1v{i}",
                                 name=f"p1v{i}_{c}") for i in range(nvp)]
                for g in range(NKT // KG):
                    gsl = slice(g * KG * P, (g + 1) * KG * P)
                    if c == 0:
                        for w_s, wsrc in ((wq_s, wqt), (wk_s, wkt),
                                          (wv_s, wvt)):
                            rr[rri % 3].dma_start(
                                out=w_s[:, g * KG:(g + 1) * KG, :],
                                in_=wsrc[gsl, :].rearrange(
                                    "(k p) m -> p k m", p=P))
                            rri += 1
                    xt4 = xt_pool.tile([P, KG, tchunk], FR, tag="xt")
                    if c == 0:
                        xq = rr[rri % 3]
                        rri += 1
                    else:
                        xq = rr[g % 2]   # sync/gpsimd; scalar has cos/sin
                    xq.dma_start(
                        out=xt4[:],
                        in_=xt[gsl, tsl].rearrange("(k p) t -> p k t", p=P))
                    for kk in range(KG):
                        kt = g * KG + kk
                        fl = dict(start=(kt == 0), stop=(kt == NKT - 1))
                        for wi, w_s in enumerate((wq_s, wk_s)):
                            for h in range(HPC):
                                msl = slice(h * P, (h + 1) * P)
                                nc.tensor.matmul(qk_ps[wi * HPC + h][:],
                                                 mm(w_s[:, kt, msl]),
                                                 mm(xt4[:, kk, :]), **fl)
                        for ts_ in range(nvp):
                            ssl = slice(ts_ * P, (ts_ + 1) * P)
                            nc.tensor.matmul(v_ps[ts_][:],
                                             mm(xt4[:, kk, ssl]),
                                             mm(wv_s[:, kt, :]), **fl)

                # evacuate the q/k PSUM accumulators to SBUF on the (idle)
                # ACT engine: the A/B PSUM slots recycle ~1us after the
                # chunk's last matmul instead of waiting for the 11us DVE
                # RoPE chain to read them
                qc = qkc_pool.tile([P, HPC, 512], F32, tag="qc", name=f"qc_{c}")
                kc = qkc_pool.tile([P, HPC, 512], F32, tag="kc", name=f"kc_{c}")
                nc.scalar.copy(out=qc[:], in_=q_ps[:])
                nc.scalar.copy(out=kc[:], in_=k_ps[:])
                qk_sb = [qc[:, 0, :tchunk], qc[:, 1, :tchunk],
                         kc[:, 0, :tchunk], kc[:, 1, :tchunk]]

                nc.scalar.dma_start(out=cos_t[:], in_=cost[:, tsl])
                nc.scalar.dma_start(out=sin_t[:], in_=sint[:, tsl])
                if c == 1:
                    # phase-2 constants: after the startup burst, long
                    # before first use (~200us)
                    nc.scalar.dma_start(out=mask_s[:],
                                        in_=masks.rearrange("r p n -> p r n"))
                    nc.scalar.dma_start(
                        out=wo_s[:],
                        in_=wot.rearrange("(mt p) o -> p mt o", p=P))

                # rope eviction: dest = sb*cos + shuffle(sb)*sin_eff
                for wi, dest in ((0, qr), (1, kr)):
                    for h in range(HPC):
                        sb = qk_sb[wi * HPC + h]
                        shuf = rtmp_pool.tile([P, tchunk], F32, tag="shuf")
                        dst = dest[h][c][:, :]
                        nc.vector.stream_shuffle(out=shuf[:], in_=sb,
                                                 mask=SWAP_MASK)
                        nc.vector.tensor_mul(out=dst, in0=sb, in1=cos_t[:])
                        nc.vector.tensor_mul(out=shuf[:], in0=shuf[:], in1=sin_t[:])
                        nc.vector.tensor_add(out=dst, in0=dst, in1=shuf[:])

                # v eviction: token-major
                for ts_ in range(nvp):
                    nc.vector.tensor_copy(out=vv[:, c * nvp + ts_, :],
                                          in_=v_ps[ts_][:])

        # -------- phase 2+3: attention with interleaved output proj -------
        # Software-pipelined: QK for tile jt+1 issues before colsum/AV of jt,
        # and both heads' exp runs as ONE wide ACT op over a 2-bank PSUM
        # tile, so ACT latency never blocks the PE stream.
        with (
            tc.tile_pool(name="outp", bufs=1) as out_pool,
            tc.tile_pool(name="exps", bufs=8) as exps_pool,
            tc.tile_pool(name="rcp", bufs=2) as rcp_pool,
            tc.tile_pool(name="p2cs", bufs=1, space="PSUM") as p2cs,
            tc.tile_pool(name="p2av", bufs=1, space="PSUM") as p2av,
        ):
            outT = [out_pool.tile([P, t], FR, tag=f"outT{h}", name=f"outT{h}")
                    for h in range(HPC)]
            oqs = [nc.sync, nc.gpsimd, nc.scalar]   # out-DMA queue rotation

            def drain_one(pend):
                (pes, plo, pw, pfl, pjt, ctx_) = pend.pop(0)
                (bb_, cs_l, av_l, isl_, c_) = ctx_
                for h in range(HPC):
                    nc.tensor.matmul(cs_l[h][:, plo:], mm(ones_s[:]),
                                     mm(pes[:, h, :pw]), **pfl)
                    nc.tensor.matmul(av_l[h][:, plo:],
                                     mm(vv[:, bb_ * (s // P) + pjt,
                                           h * P:(h + 1) * P]),
                                     mm(pes[:, h, :pw]), **pfl)
                if not pfl["stop"]:
                    return
                # chunk epilogue: normalize + output projection
                for h in range(HPC):
                    rcp = rcp_pool.tile([P, tchunk], F32, tag="rcp",
                                        name=f"rcp{h}_{bb_}_{c_}")
                    nc.vector.reciprocal_approx_fast(out=rcp[:], in_=cs_l[h][:])
                    nc.vector.tensor_mul(out=outT[h][:, isl_], in0=av_l[h][:],
                                         in1=rcp[:])
                wo_pools = [p2cs, p2cs, p2av, p2av]
                wo_tags = ["cs0", "cs1", "av0", "av1"]
                wi_ = 0
                for tt_ in range(tchunk // P):
                    tt0 = isl_.start + tt_ * P
                    ttsl = slice(tt0, tt0 + P)
                    ev = ev_pool.tile([P, H], FR, tag="ev",
                                      name=f"ev_{tt0}")
                    for oc in range(H // 512):
                        osl = slice(oc * 512, (oc + 1) * 512)
                        ps = wo_pools[wi_ % 4].tile(
                            [P, 512], F32, tag=wo_tags[wi_ % 4],
                            name=f"wo_{tt0}_{oc}")
                        for h in range(HPC):
                            nc.tensor.matmul(ps[:],
                                             mm(outT[h][:, ttsl]),
                                             mm(wo_s[:, h, osl]),
                                             start=(h == 0),
                                             stop=(h == HPC - 1))
                        nc.vector.tensor_copy(out=ev[:, osl], in_=ps[:])
                        wi_ += 1
                    # one wide DMA per token tile: 4KB descriptors, and
                    # 4 (not 16) DMAs per epilogue keeps each queue under
                    # its semaphore-reuse depth
                    oqs[tt_ % 3].dma_start(out=out[ttsl, :], in_=ev[:])

            pend = []
            for bb in range(b):
                for c in range(ich):
                    isl = slice(bb * s + c * tchunk, bb * s + (c + 1) * tchunk)
                    njt = r_mask * (c + 1)   # visible j-tiles
                    cs_ps = [p2cs.tile([P, tchunk], F32, tag=f"cs{h}",
                                       name=f"cs{h}_{bb}_{c}") for h in range(HPC)]
                    av_ps = [p2av.tile([P, tchunk], F32, tag=f"av{h}",
                                       name=f"av{h}_{bb}_{c}") for h in range(HPC)]
                    cctx = (bb, cs_ps, av_ps, isl, c)
                    for jt in range(njt):
                        jc = bb * ich + jt // r_mask
                        jlo = (jt % r_mask) * P
                        ri = jt - r_mask * c
                        lo = max(ri, 0) * P
                        w = tchunk - lo
                        fl = dict(start=(jt == 0), stop=(jt == njt - 1))
                        sc = ab_pool.tile([P, HPC, 512], F32,
                                          tag=("A", "B")[jt % 2],
                                          name=f"sc_{bb}_{c}_{jt}")
                        for h in range(HPC):
                            nc.tensor.matmul(
                                sc[:, h, :w],
                                mm(kr[h][jc][:, jlo:jlo + P]),
                                mm(qr[h][bb * ich + c][:, lo:]),
                                start=True, stop=True)
                        es = exps_pool.tile([P, HPC, tchunk], FR, tag="es",
                                            name=f"es_{bb}_{c}_{jt}")
                        nc.scalar.activation(out=es[:, :, :w], in_=sc[:, :, :w],
                                             func=mybir.ActivationFunctionType.Exp,
                                             scale=float(SCALE))
                        if ri >= 0:  # diagonal-crossing tile
                            mb = mask_s[:, ri, lo:].unsqueeze(1).broadcast_to(
                                [P, HPC, w])
                            nc.vector.tensor_mul(out=es[:, :, :w],
                                                 in0=es[:, :, :w], in1=mb)
                        pend.append((es, lo, w, fl, jt, cctx))
                        if len(pend) > 2:
                            drain_one(pend)
            while pend:
                drain_one(pend)

    nc.compile()
    return nc


def make_in_maps(hidden_states, cos, sin, Wq, Wk, Wv, Wo, s=S, b=B, tchunk=512):
    t = b * s
    tchunk = min(tchunk, t)
    hs = np.asarray(hidden_states, np.float32).reshape(t, H)
    xt = np.ascontiguousarray(hs.T)
    cos2 = np.asarray(cos, np.float32).reshape(s, HD)
    sin2 = np.asarray(sin, np.float32).reshape(s, HD)
    cosP = np.ascontiguousarray(np.tile(cos2[:, PERM].T, (1, b)))
    sign = np.where(PERM < 64, -1.0, 1.0).astype(np.float32)[:, None]
    sinP = np.ascontiguousarray(np.tile(sin2[:, PERM].T * sign, (1, b)))
    masks_bf = build_masks(tchunk).astype(BF16)
    xt_bf = xt.astype(BF16)
    Wq, Wk, Wv, Wo = (np.asarray(w, np.float32) for w in (Wq, Wk, Wv, Wo))

    in_maps = []
    for c in range(NCORES):
        rows = np.concatenate([(HPC * c + hh) * HD + PERM for hh in range(HPC)])
        sl = slice(c * M, (c + 1) * M)
        in_maps.append({
            "xt": xt_bf,
            "wqt": np.ascontiguousarray(Wq[rows, :].T).astype(BF16),
            "wkt": np.ascontiguousarray(Wk[rows, :].T).astype(BF16),
            "wvt": np.ascontiguousarray(Wv[sl, :].T).astype(BF16),
            "wot": np.ascontiguousarray(Wo[:, sl].T).astype(BF16),
            "cost": cosP,
            "sint": sinP,
            "masks": masks_bf,
        })
    return in_maps


_CACHED_NC = None
_LAST_RESULTS = None


def kernel(hidden_states, cos, sin, Wq, Wk, Wv, Wo):
    global _CACHED_NC, _LAST_RESULTS
    in_maps = make_in_maps(hidden_states, cos, sin, Wq, Wk, Wv, Wo)
    if _CACHED_NC is None:
        _CACHED_NC = build_nc()
    res = run_bass_kernel_spmd(_CACHED_NC, in_maps, core_ids=list(range(NCORES)))
    _LAST_RESULTS = res
    acc = np.zeros((B * S, H), np.float32)
    for r in res.results:
        acc += r["out"].astype(np.float32)
    return acc.reshape(B, S, H)
